# revision 8
# baseline (speedup 1.0000x reference)
"""Trainium2 Bass kernel for the IRNN spatial-recurrence module.

Computation (the attention branch of the reference is dead code and skipped):
    out = cin_w @ x                                  (1x1 conv, 512->512)
    out = c2_w @ concat(IRNN1(out))                  (2048->512)
    out = relu(c3_w @ concat(IRNN2(out)))            (2048->512)
where IRNN(x) = 4 directional relu-recurrences along H/W with per-channel
recurrence weight w (== 1.0 for these inputs) and bias b.

Sharding: 8 cores = 4 batches x 2 channel-halves. Each core computes its
256 channels of cin/IRNN1, a full-512-row partial of c2 over its 1024
concat columns, pairwise ReduceScatter to sum partials and keep its own
256 rows, IRNN2 on those, then the same for c3.

The relu-recurrence s' = max(s + (x+b), 0) maps onto the DVE
tensor_tensor_scan instruction (op0=add, op1=max) over a padded layout:
each 64-pixel segment is prefixed with a -1e30 separator so one scan
instruction handles all 64 independent lanes of a direction, with the
separator forcing the carried state to 0 at each segment start.
"""
import sys
sys.path.insert(0, '/opt/trn_rl_repo')

import numpy as np
import concourse.bass as bass
import concourse.mybir as mybir
import concourse.tile as tile

B, C, H, W = 4, 512, 64, 64
PX = H * W          # 4096
CO = C // 2         # 256 channels per core
NCHUNK = 8          # pixel chunks of 512
CH = PX // NCHUNK   # 512 px per chunk
ROWS = H // NCHUNK  # 8 h-rows per chunk
NEG = -1.0e30
DIRS = ["u", "r", "d", "l"]


# ---------------------------------------------------------------------------
# wait legalization: this walrus build supports ~2 sync commands per
# instruction (1 for the scan / CTRL_NO structs). Move excess waits onto
# injected same-engine NoOps placed immediately before the instruction.
def _wait_budget(inst) -> int:
    n_upd = 0
    si = inst.sync_info
    if si is not None:
        n_upd = len(si.on_update)
    if isinstance(inst, mybir.InstTensorScalarPtr) and getattr(
            inst, "is_tensor_tensor_scan", False):
        total = 1
    elif isinstance(inst, (mybir.InstNoOp, mybir.InstDrain)):
        total = 1
    else:
        total = 2
    return max(0, total - n_upd)


def split_excess_waits(nc: bass.Bass) -> int:
    n_split = 0
    for f in nc.m.functions:
        for blk in f.blocks:
            insts = blk.instructions
            i = 0
            while i < len(insts):
                inst = insts[i]
                si = inst.sync_info
                if si is None or not si.on_wait:
                    i += 1
                    continue
                budget = _wait_budget(inst)
                waits = list(si.on_wait)
                if len(waits) <= budget:
                    i += 1
                    continue
                excess, keep = waits[:len(waits) - budget], waits[len(waits) - budget:]
                for w in excess:
                    nop = mybir.InstNoOp(name=f"{inst.name}-wn{n_split}")
                    nop.engine = inst.engine
                    nop.sync_info = mybir.SyncInfo(on_wait=[w], on_update=[])
                    insts.insert(i, nop)
                    i += 1
                    n_split += 1
                inst.sync_info = mybir.SyncInfo(
                    on_wait=keep, on_update=list(si.on_update))
                i += 1
    return n_split


# ---------------------------------------------------------------------------
def build_kernel():
    f32, f32r = mybir.dt.float32, mybir.dt.float32r
    nc = bass.Bass()
    x_in = nc.declare_dram_parameter("x", [C, PX], f32r, isOutput=False)
    cin_wT = nc.declare_dram_parameter("cin_wT", [C, CO], f32r, isOutput=False)
    c2_wT = nc.declare_dram_parameter("c2_wT", [4 * CO, C], f32r, isOutput=False)
    c3_wT = nc.declare_dram_parameter("c3_wT", [4 * CO, C], f32r, isOutput=False)
    # bias rows: [blk1: b_u,b_r,b_d,b_l,-b_u,-b_r,-b_d,-b_l, blk2: same]
    b_in = nc.declare_dram_parameter("bias", [16, CO], f32, isOutput=False)
    out_p = nc.declare_dram_parameter("out", [CO, PX], f32, isOutput=True)

    groups = [[0, 1], [2, 3], [4, 5], [6, 7]]

    from contextlib import ExitStack
    with tile.TileContext(nc) as tc, ExitStack() as es:
        const = es.enter_context(tc.tile_pool(name="const", bufs=1))
        wpool = es.enter_context(tc.tile_pool(name="w", bufs=1))
        xpool = es.enter_context(tc.tile_pool(name="x", bufs=2))
        bufp = es.enter_context(tc.tile_pool(name="scanbuf", bufs=1))
        ldp = es.enter_context(tc.tile_pool(name="loadback", bufs=2))
        outp = es.enter_context(tc.tile_pool(name="outstage", bufs=2))
        psP = es.enter_context(tc.tile_pool(name="ps", bufs=6, space="PSUM"))
        dram = es.enter_context(tc.tile_pool(name="dram", bufs=1, space="DRAM"))

        # ---- constants -------------------------------------------------
        bt = {}  # bias tiles [128,1] per (block, dir, sign, ctile m)
        for blk in (0, 1):
            for i, d in enumerate(DIRS):
                for s, sgn in enumerate(["p", "n"]):
                    for m in (0, 1):
                        t = const.tile([128, 1], f32,
                                       tag=f"b{blk}{d}{sgn}{m}")
                        nc.sync.dma_start(
                            t[:],
                            b_in[8 * blk + 4 * s + i,
                                 128 * m:128 * (m + 1)].unsqueeze(1))
                        bt[(blk, d, sgn, m)] = t
        zcol = const.tile([128, 1], f32)
        nc.vector.memset(zcol[:], 0.0)
        zbc = zcol[:].broadcast_to([128, H * (W + 1)])

        cin_k = []
        for k in range(4):
            t = wpool.tile([128, CO], f32r, tag=f"cin{k}")
            nc.sync.dma_start(t[:], cin_wT[128 * k:128 * (k + 1), :])
            cin_k.append(t)
        c2_k, c3_k = [], []
        for k in range(8):
            t = wpool.tile([128, C], f32r, tag=f"c2_{k}")
            nc.sync.dma_start(t[:], c2_wT[128 * k:128 * (k + 1), :])
            c2_k.append(t)
        for k in range(8):
            t = wpool.tile([128, C], f32r, tag=f"c3_{k}")
            nc.sync.dma_start(t[:], c3_wT[128 * k:128 * (k + 1), :])
            c3_k.append(t)

        # DRAM bounce buffers for the two collectives
        p2 = dram.tile([C, PX], f32)   # c2 partial (512 rows)
        s2 = dram.tile([CO, PX], f32)  # c2 summed, my rows
        p3 = dram.tile([C, PX], f32)
        s3 = dram.tile([CO, PX], f32)

        # ---- helpers ---------------------------------------------------
        def stage_dir_copies(bufs, psum_or_sb, blk, m, j):
            """4 biased copies of one [128, ROWS, W] chunk into the padded
            direction buffers. src AP shaped [128, ROWS, W]."""
            src = psum_or_sb
            r0 = ROWS * j
            nc.scalar.add(bufs["r"][m][:, r0:r0 + ROWS, 1:W + 1],
                          src, bt[(blk, "r", "p", m)][:])
            nc.scalar.add(bufs["l"][m][:, r0:r0 + ROWS, 1:W + 1][:, :, ::-1],
                          src, bt[(blk, "l", "p", m)][:])
            nc.scalar.add(
                bufs["d"][m][:, :, 1 + r0:1 + r0 + ROWS].transpose([0, 2, 1]),
                src, bt[(blk, "d", "p", m)][:])
            nc.scalar.add(
                bufs["u"][m][:, :, W + 1 - r0 - ROWS:W + 1 - r0]
                [:, :, ::-1].transpose([0, 2, 1]),
                src, bt[(blk, "u", "p", m)][:])

        def finish_scans(bufs, blk):
            for d in DIRS:
                for m in (0, 1):
                    buf = bufs[d][m]
                    nc.vector.tensor_scalar_add(
                        buf[:, :, 1:2], buf[:, :, 1:2], bt[(blk, d, "n", m)][:])
                    flat = buf[:].rearrange("p a b -> p (a b)")
                    nc.vector.tensor_tensor_scan(
                        flat, flat, zbc, 0.0,
                        mybir.AluOpType.add, mybir.AluOpType.max)
                    nc.vector.memset(buf[:, :, 1:2].bitcast(mybir.dt.float32), 0.0)

        def rhs_ap(bufs, d, m, j):
            r0 = ROWS * j
            if d == "r":
                return bufs["r"][m][:, r0:r0 + ROWS, 1:W + 1]
            if d == "l":
                return bufs["l"][m][:, r0:r0 + ROWS, 1:W + 1][:, :, ::-1]
            if d == "d":
                return bufs["d"][m][:, :, 1 + r0:1 + r0 + ROWS].transpose([0, 2, 1])
            return bufs["u"][m][:, :, W + 1 - r0 - ROWS:W + 1 - r0] \
                [:, :, ::-1].transpose([0, 2, 1])

        def alloc_bufs(blk):
            bufs = {d: [] for d in DIRS}
            for d in DIRS:
                for m in (0, 1):
                    buf = bufp.tile([128, H, W + 1], f32r, tag=f"buf_{d}{m}")
                    nc.vector.memset(buf[:, :, 0:1].bitcast(mybir.dt.float32), NEG)
                    bufs[d].append(buf)
            return bufs

        # ---- stage A: cin GEMM + IRNN1 staging ------------------------
        bufs1 = alloc_bufs(0)
        for j in range(NCHUNK):
            xk = []
            for k in range(4):
                t = xpool.tile([128, CH], f32r, tag=f"xk{k}")
                nc.sync.dma_start(
                    t[:], x_in[128 * k:128 * (k + 1), CH * j:CH * (j + 1)])
                xk.append(t)
            for m in (0, 1):
                acc = psP.tile([128, CH], f32, tag="ps")
                for k in range(4):
                    nc.tensor.matmul(acc[:], cin_k[k][:, 128 * m:128 * (m + 1)],
                                     xk[k][:], start=(k == 0), stop=(k == 3))
                src = acc[:].rearrange("p (a b) -> p a b", a=ROWS)
                stage_dir_copies(bufs1, src, 0, m, j)
        finish_scans(bufs1, 0)

        # ---- stage B: c2 partial GEMM -> DRAM -> RS -------------------
        def partial_gemm(bufs, wk, pdram):
            for j in range(NCHUNK):
                for m2 in range(4):
                    acc = psP.tile([128, CH], f32, tag="ps")
                    kt = 0
                    for di, d in enumerate(DIRS):
                        for m in (0, 1):
                            nc.tensor.matmul(
                                acc[:],
                                wk[kt][:, 128 * m2:128 * (m2 + 1)],
                                rhs_ap(bufs, d, m, j),
                                start=(kt == 0), stop=(kt == 7))
                            kt += 1
                    st = outp.tile([128, CH], f32, tag="pstage")
                    nc.scalar.copy(st[:], acc[:])
                    nc.sync.dma_start(
                        pdram[128 * m2:128 * (m2 + 1), CH * j:CH * (j + 1)],
                        st[:])

        partial_gemm(bufs1, c2_k, p2)
        nc.gpsimd.collective_compute(
            "ReduceScatter", mybir.AluOpType.add, replica_groups=groups,
            ins=[p2[:]], outs=[s2[:]])

        # ---- stage B2: IRNN2 staging from s2 --------------------------
        bufs2 = alloc_bufs(1)
        for j in range(NCHUNK):
            for m in (0, 1):
                t = ldp.tile([128, CH], f32, tag="ld")
                nc.sync.dma_start(
                    t[:], s2[128 * m:128 * (m + 1), CH * j:CH * (j + 1)])
                src = t[:].rearrange("p (a b) -> p a b", a=ROWS)
                stage_dir_copies(bufs2, src, 1, m, j)
        finish_scans(bufs2, 1)

        # ---- stage C: c3 partial GEMM -> DRAM -> RS -> relu -> out ----
        partial_gemm(bufs2, c3_k, p3)
        nc.gpsimd.collective_compute(
            "ReduceScatter", mybir.AluOpType.add, replica_groups=groups,
            ins=[p3[:]], outs=[s3[:]])

        for j in range(NCHUNK):
            for m in (0, 1):
                t = ldp.tile([128, CH], f32, tag="ld2")
                nc.sync.dma_start(
                    t[:], s3[128 * m:128 * (m + 1), CH * j:CH * (j + 1)])
                o = outp.tile([128, CH], f32, tag="ostage")
                nc.scalar.activation(o[:], t[:],
                                     mybir.ActivationFunctionType.Relu)
                nc.sync.dma_start(
                    out_p[128 * m:128 * (m + 1), CH * j:CH * (j + 1)], o[:])

    split_excess_waits(nc)
    return nc


_NC_CACHE = None


def _get_nc():
    global _NC_CACHE
    if _NC_CACHE is None:
        _NC_CACHE = build_kernel()
    return _NC_CACHE


# ---------------------------------------------------------------------------
def _reference_np(inputs):
    """Numpy fallback (exact port of the reference main path)."""
    x = inputs["x"]

    def conv1x1(x, w):
        return np.einsum("oi,bihw->bohw", w, x)

    def scan_dir(x, w, b, axis, reverse):
        xs = np.moveaxis(x, axis, 1)  # [B, L, C, other]
        if reverse:
            xs = xs[:, ::-1]
        L = xs.shape[1]
        ys = np.zeros_like(xs)
        st = np.maximum(xs[:, 0], 0.0)
        for t in range(1, L):
            st = np.maximum(st * w[:, None] + b[:, None] + xs[:, t], 0.0)
            ys[:, t] = st
        if reverse:
            ys = ys[:, ::-1]
        return np.moveaxis(ys, 1, axis)

    def irnn(x, tag):
        outs = []
        for d, axis, rev in (("u", 2, True), ("r", 3, False),
                             ("d", 2, False), ("l", 3, True)):
            outs.append(scan_dir(x, inputs[f"{tag}_w{d}"],
                                 inputs[f"{tag}_b{d}"], axis, rev))
        return np.concatenate(outs, axis=1)

    out = conv1x1(x, inputs["cin_w"])
    out = conv1x1(irnn(out, "i1"), inputs["c2_w"])
    out = np.maximum(conv1x1(irnn(out, "i2"), inputs["c3_w"]), 0.0)
    return out.astype(np.float32)


def _build_in_maps(inputs):
    x = np.asarray(inputs["x"], np.float32)
    cin_w = np.asarray(inputs["cin_w"], np.float32)
    c2_w = np.asarray(inputs["c2_w"], np.float32)
    c3_w = np.asarray(inputs["c3_w"], np.float32)

    in_maps = []
    for r in range(8):
        b, g = r // 2, r % 2
        gs = slice(g * CO, (g + 1) * CO)
        cols = np.concatenate(
            [np.arange(d * C + g * CO, d * C + (g + 1) * CO) for d in range(4)])
        bias = np.empty((16, CO), np.float32)
        for blk, tag in enumerate(("i1", "i2")):
            for i, d in enumerate(DIRS):
                bv = np.asarray(inputs[f"{tag}_b{d}"], np.float32)[gs]
                bias[8 * blk + i] = bv
                bias[8 * blk + 4 + i] = -bv
        in_maps.append({
            "x": np.ascontiguousarray(x[b].reshape(C, PX)),
            "cin_wT": np.ascontiguousarray(cin_w[gs, :].T),
            "c2_wT": np.ascontiguousarray(c2_w[:, cols].T),
            "c3_wT": np.ascontiguousarray(c3_w[:, cols].T),
            "bias": bias,
        })
    return in_maps


def kernel(**inputs) -> np.ndarray:
    ws = [inputs[f"{t}_w{d}"] for t in ("i1", "i2") for d in ("u", "r", "d", "l")]
    if not all(np.all(np.asarray(w) == 1.0) for w in ws):
        return _reference_np(inputs)

    from concourse.bass_utils import run_bass_kernel_spmd

    nc = _get_nc()
    in_maps = _build_in_maps(inputs)
    res = run_bass_kernel_spmd(nc, in_maps, list(range(8)))
    out = np.empty((B, C, H, W), np.float32)
    for r in range(8):
        b, g = r // 2, r % 2
        out[b, g * CO:(g + 1) * CO] = res.results[r]["out"].reshape(CO, H, W)
    return out
